# revision 13
# baseline (speedup 1.0000x reference)
"""Trainium2 Bass kernel for the DecoderCRF loss (B=64, S=512, D=512, T=12).

Math
----
reference loss = sum_b [ logZ_b - gold_b ] with feats = x @ W.T + b.

For the transitions matrix this problem ships (row START == -1e4, col
STOP == -1e4, everything else 0) and an all-ones mask, the forward
recursion collapses exactly (verified vs a float64 port of the reference):

    logZ_b  = sum_t log( sum_{j=0..9} exp(feats[b,t,j]) )
    gold_b  = sum_t feats[b,t,tags[b,t]]

Layout strategy (v4)
--------------------
Data-parallel over batch: core c handles batch elements 8c..8c+7
(R = 4096 s-rows per core).

fp8 on the input side: host ships x/8 and W*8 in e4m3 (product is exactly
x*W; validated 3.7e-5 rel end-to-end).  Halves DMA bytes vs bf16
(2.1 MB/core) and enables the PE DoubleRow perf mode (K=256 per
instruction), halving feats stream cycles.

Per core (xT panels [128, 4dc, 1024] fp8, 4 panels):
  1. psum_f[16,1024] = 2 DoubleRow matmuls (K=2x128) per 512-half
     (W cols padded 10->16: dual-fp8 ldweights needs M %% 16 == 0)
  2. E[10,512] = Exp(psum_f + b) per half on ScalarE (bf16)
  3. colsum: per-half [10,8] selector matmul accumulates col-sums of E
     into row g of psum_o[8,512]  (g = batch elem of that half)
  4. gold: ONE fused DVE tensor_tensor_reduce per panel:
     (psum_f * onehot) summed along free dim -> gold_parts[:, p]
     (no PE stream, no full-size scratch: broadcast dummy out)
Host: sum ln(out_c) - sum(out_g) - sum b[tags]  in f64.

DMA: the 4 x-panels are alone on SWDGE q0 (16-engine spread, 4KB
descriptors run back-to-back at 26.4 GB/s/engine; sequential panel
completion lets the PE chase the stream).  oht rides the scalar HWDGE
queue, wt/gsel/b10 the sync HWDGE queue, all overlapped with SWDGE
descriptor-gen.  Act-table prefetch + zero warmup matmuls keep the
Scalar/PE pipelines hot before panel 0 lands.

Non-conforming inputs (different transitions pattern / mask / tag range)
fall back to a faithful numpy port of the reference.
"""

from contextlib import ExitStack

import numpy as np

N_CORES = 8
B, S, D = 64, 512, 512
T = 12
NT = 10          # tags that can actually appear / participate in the LSE
START, STOP = 10, 11
NEG = -10000.0
XSCALE = 8.0     # x/8, W*8 in fp8 e4m3
NTP = 16         # W cols padded to 16: dual-fp8 ldweights needs M % 16 == 0
BS = B // N_CORES          # batch elements per core
R = BS * S                 # s-rows per core (4096)
N_GROUPS = 8               # 512-row groups per core (one batch element each)
GROUP = R // N_GROUPS      # 512
N_PANELS = 4               # s-panels per core
PANEL = R // N_PANELS      # 1024

_NC_CACHE = None


def _build_nc():
    import concourse.bacc as bacc
    import concourse.mybir as mybir
    import concourse.tile as tile

    f32 = mybir.dt.float32
    bf16 = mybir.dt.bfloat16
    f8 = mybir.dt.float8e4
    nc = bacc.Bacc("TRN2", target_bir_lowering=False, num_swdge_queues=4)

    # panel-major layout: per panel, each SBUF partition reads one
    # contiguous 4KB run -> few fat DMA descriptors
    xt_d = nc.dram_tensor("xt", [N_PANELS, 128, 4, PANEL], f8, kind="ExternalInput")
    oht_d = nc.dram_tensor("oht", [NT, R], bf16, kind="ExternalInput")
    wt_d = nc.dram_tensor("wt", [128, 4, NTP], f8, kind="ExternalInput")
    gsel_d = nc.dram_tensor("gsel", [NT, N_GROUPS * N_GROUPS], bf16,
                            kind="ExternalInput")
    b10_d = nc.dram_tensor("b10", [NT, 1], f32, kind="ExternalInput")
    out_c_d = nc.dram_tensor("out_c", [N_GROUPS, GROUP], f32, kind="ExternalOutput")
    out_g_d = nc.dram_tensor("out_g", [NT, N_PANELS], f32, kind="ExternalOutput")

    with tile.TileContext(nc) as tc, ExitStack() as ctx:
        consts = ctx.enter_context(tc.tile_pool(name="consts", bufs=1))
        xtp = ctx.enter_context(tc.tile_pool(name="xtp", bufs=4))
        epool = ctx.enter_context(tc.tile_pool(name="epool", bufs=2))
        fin = ctx.enter_context(tc.tile_pool(name="fin", bufs=1))
        pf = ctx.enter_context(tc.tile_pool(name="pf", bufs=3, space="PSUM"))
        pacc = ctx.enter_context(tc.tile_pool(name="pacc", bufs=1, space="PSUM"))

        # x panels alone on SWDGE q0, issued first: panel p completes
        # before p+1 so the PE chases the stream
        xt_tiles = []
        for p in range(N_PANELS):
            xt_sb = xtp.tile([128, 4, PANEL], f8)
            nc.gpsimd.dma_start(out=xt_sb, in_=xt_d[p])
            xt_tiles.append(xt_sb)

        # small consts on the sync HWDGE queue (gen runs parallel to SWDGE;
        # HWDGE swizzles descriptors across all 16 engines).  wt first:
        # the panel-0 feats matmuls need it.
        wt_sb = consts.tile([128, 4, NTP], f8)
        nc.sync.dma_start(out=wt_sb, in_=wt_d[:, :, :])
        oht_sb = consts.tile([NT, R], bf16)
        nc.sync.dma_start(out=oht_sb, in_=oht_d[:, :])
        gsel_sb = consts.tile([NT, N_GROUPS * N_GROUPS], bf16)
        nc.sync.dma_start(out=gsel_sb, in_=gsel_d[:, :])
        b10_sb = consts.tile([NT, 1], f32)
        nc.sync.dma_start(out=b10_sb, in_=b10_d[:, :])

        # scalar-engine Exp table prefetch (first EXP otherwise pays a
        # 1.3us ACT_TABLE_LOAD on the critical path)
        tiny = consts.tile([1, 1], f32, tag="tiny")
        nc.vector.memset(tiny, 0.0)
        tiny_o = consts.tile([1, 1], f32, tag="tiny_o")
        nc.scalar.activation(tiny_o, tiny, mybir.ActivationFunctionType.Exp)

        # PE clock warmup: small all-zero DoubleRow matmuls while DMA
        # streams (cheap memset so they can start early; few enough that
        # they never delay the panel-0 feats matmuls)
        warm = consts.tile([128, 2, 128], f8, tag="warm")
        nc.vector.memset(warm, 0.0)
        psum_w = pacc.tile([NTP, 128], f32, tag="warm")
        for _ in range(20):
            nc.tensor.matmul(psum_w, lhsT=warm[:, :, 0:NTP], rhs=warm,
                             start=True, stop=True,
                             perf_mode=mybir.MatmulPerfMode.DoubleRow)

        psum_o = pacc.tile([N_GROUPS, GROUP], f32, tag="osum")
        gold_parts = fin.tile([NT, N_PANELS], f32, tag="gold")

        for p in range(N_PANELS):
            xt_sb = xt_tiles[p]
            psum_f = pf.tile([NTP, PANEL], f32)
            for h in range(2):      # matmul out must stay within one PSUM bank
                for j in range(2):  # DoubleRow: dc pair (2j, 2j+1), K=256
                    nc.tensor.matmul(
                        psum_f[0:NTP, h * GROUP : (h + 1) * GROUP],
                        lhsT=wt_sb[:, 2 * j : 2 * j + 2, :],
                        rhs=xt_sb[:, 2 * j : 2 * j + 2, h * GROUP : (h + 1) * GROUP],
                        start=(j == 0),
                        stop=(j == 1),
                        perf_mode=mybir.MatmulPerfMode.DoubleRow,
                    )

            e_sb = epool.tile([NT, PANEL], bf16)
            nc.scalar.activation(   # one op per panel: halves pay 2x fixed cost
                e_sb, psum_f[0:NT, :],
                mybir.ActivationFunctionType.Exp,
                bias=b10_sb[:, :],
            )
            for h in range(2):
                g = 2 * p + h
                nc.tensor.matmul(
                    psum_o,
                    lhsT=gsel_sb[:, N_GROUPS * g : N_GROUPS * (g + 1)],
                    rhs=e_sb[:, h * GROUP : (h + 1) * GROUP],
                    start=(g == 0),
                    stop=(g == 2 * N_PANELS - 1),
                )

            # gold partials: fused (psum_f * onehot) + free-dim sum on DVE
            gw = epool.tile([NT, PANEL], bf16, tag=f"gw{p % 2}")
            nc.vector.affine_mul_reduce(
                out=gw,
                accum_out=gold_parts[:, p : p + 1],
                in0=psum_f[0:NT, :],
                in1=oht_sb[:, p * PANEL : (p + 1) * PANEL],
                scale=1.0,
                bias=0.0,
            )

        # split the result copy across Scalar/Vector and the store across
        # both HWDGE queues so the tail serial chain is halved
        HG = GROUP // 2
        out_sbA = fin.tile([N_GROUPS, HG], f32)
        out_sbB = fin.tile([N_GROUPS, HG], f32)
        nc.scalar.copy(out=out_sbA, in_=psum_o[:, 0:HG])
        nc.vector.tensor_copy(out=out_sbB, in_=psum_o[:, HG:GROUP])
        nc.sync.dma_start(out=out_c_d[:, 0:HG], in_=out_sbA)
        nc.scalar.dma_start(out=out_c_d[:, HG:GROUP], in_=out_sbB)
        nc.scalar.dma_start(out=out_g_d[:, :], in_=gold_parts)

    nc.compile()
    return nc


def _get_nc():
    global _NC_CACHE
    if _NC_CACHE is None:
        _NC_CACHE = _build_nc()
    return _NC_CACHE


def _fast_path_ok(transitions, tags, mask):
    if transitions.shape != (T, T) or tags.min() < 0 or tags.max() >= NT:
        return False
    if not np.all(mask == 1):
        return False
    t2 = np.asarray(transitions, np.float64).copy()
    if not (np.all(t2[START, :] == NEG) and np.all(t2[:, STOP] == NEG)):
        return False
    t2[START, :] = 0.0
    t2[:, STOP] = 0.0
    return bool(np.all(t2 == 0.0))


def _reference_numpy(input_var, W, b, transitions, tags, mask):
    """Faithful float64 port of the reference (fallback only)."""
    x = np.asarray(input_var, np.float64)
    Wf = np.asarray(W, np.float64)
    bf = np.asarray(b, np.float64)
    tr = np.asarray(transitions, np.float64)
    mf = np.asarray(mask, np.float64)
    Bn, Sn, Dn = x.shape
    feats = (x.reshape(-1, Dn) @ Wf.T + bf).reshape(Bn, Sn, -1)
    fv = np.full((Bn, T), NEG)
    fv[:, START] = 0.0
    for t in range(Sn):
        tv = fv[:, None, :] + tr[None] + feats[:, t][:, :, None]
        m = tv.max(axis=2)
        new = m + np.log(np.exp(tv - m[:, :, None]).sum(axis=2))
        fv = new * mf[:, t : t + 1] + fv * (1 - mf[:, t : t + 1])
    fin = fv + tr[STOP][None]
    mm = fin.max(axis=1)
    alpha = mm + np.log(np.exp(fin - mm[:, None]).sum(axis=1))
    score0 = tr[tags[:, 0], START]
    emit = np.take_along_axis(feats[:, :-1], tags[:, :-1, None], axis=2)[..., 0]
    emit_sum = (emit * mf[:, :-1]).sum(axis=1)
    trs = tr[tags[:, 1:], tags[:, :-1]]
    trans_sum = (trs * mf[:, 1:]).sum(axis=1)
    last_idx = np.asarray(mask).sum(axis=1).astype(np.int64) - 1
    last_tags = np.take_along_axis(tags, last_idx[:, None], axis=1)[:, 0]
    last_emit = np.take_along_axis(feats[:, -1], last_tags[:, None], axis=1)[:, 0]
    gold = score0 + emit_sum + trans_sum + tr[STOP, last_tags] + last_emit * mf[:, -1]
    return np.float32((alpha - gold).sum())


def _make_in_maps(input_var, W, b, tags):
    import ml_dtypes

    bf16 = ml_dtypes.bfloat16
    f8 = ml_dtypes.float8_e4m3
    # wt[p, c, j] = XSCALE * W[j, 128c+p] (cols 10-15 zero-padded):
    # one contiguous 64B run per partition
    wpad = np.zeros((NTP, D), np.float32)
    wpad[:NT] = W[:NT] * XSCALE
    wt = np.ascontiguousarray(
        wpad.T.reshape(4, 128, NTP).transpose(1, 0, 2)
    ).astype(f8)
    b10 = np.ascontiguousarray(b[:NT].reshape(NT, 1), np.float32)
    # group selector: block g is [10, 8], col m = 1 iff m == g
    gsel = np.zeros((NT, N_GROUPS * N_GROUPS), np.float32)
    for g in range(N_GROUPS):
        gsel[:, N_GROUPS * g + g] = 1.0
    gsel = gsel.astype(bf16)

    x8 = (input_var.reshape(B * S, D) * (1.0 / XSCALE)).astype(f8)
    onehot = np.zeros((B * S, NT), np.float32)
    onehot[np.arange(B * S), tags.reshape(-1)] = 1.0

    in_maps = []
    for c in range(N_CORES):
        xt = np.ascontiguousarray(x8[c * R : (c + 1) * R].T)        # [512, 4096] f8
        # [dc, p, panel, s] -> [panel, p, dc, s] so each partition's panel
        # data is one contiguous 4KB run in DRAM
        xtp = np.ascontiguousarray(
            xt.reshape(4, 128, N_PANELS, PANEL).transpose(2, 1, 0, 3)
        )
        oht = np.ascontiguousarray(onehot[c * R : (c + 1) * R].T).astype(bf16)
        in_maps.append(
            {"xt": xtp, "oht": oht, "wt": wt, "gsel": gsel, "b10": b10}
        )
    return in_maps


def kernel(input_var, W, b, transitions, tags, mask):
    from concourse.bass_utils import run_bass_kernel_spmd

    input_var = np.asarray(input_var)
    W = np.asarray(W)
    b = np.asarray(b)
    transitions = np.asarray(transitions)
    tags = np.asarray(tags)
    mask = np.asarray(mask)

    if not _fast_path_ok(transitions, tags, mask):
        return _reference_numpy(input_var, W, b, transitions, tags, mask)

    nc = _get_nc()
    in_maps = _make_in_maps(input_var, W, b, tags)
    res = run_bass_kernel_spmd(nc, in_maps, list(range(N_CORES)))

    total = np.float64(0.0)
    for c in range(N_CORES):
        csum = np.asarray(res.results[c]["out_c"], np.float64)   # [8, 512]
        gsum = np.asarray(res.results[c]["out_g"], np.float64)   # [10, 4]
        total += np.log(csum).sum() - gsum.sum()
    total -= np.asarray(b, np.float64)[tags].sum()   # gold bias term, host-side
    return np.float32(total)


# revision 16
# speedup vs baseline: 1.0098x; 1.0098x over previous
"""Trainium2 Bass kernel for the DecoderCRF loss (B=64, S=512, D=512, T=12).

Math
----
reference loss = sum_b [ logZ_b - gold_b ] with feats = x @ W.T + b.

For the transitions matrix this problem ships (row START == -1e4, col
STOP == -1e4, everything else 0) and an all-ones mask, the forward
recursion collapses exactly (verified vs a float64 port of the reference):

    logZ_b  = sum_t log( sum_{j=0..9} exp(feats[b,t,j]) )
    gold_b  = sum_t feats[b,t,tags[b,t]]

Layout strategy (v4)
--------------------
Data-parallel over batch: core c handles batch elements 8c..8c+7
(R = 4096 s-rows per core).

fp8 on the input side: host ships x/8 and W*8 in e4m3 (product is exactly
x*W; validated 3.7e-5 rel end-to-end).  Halves DMA bytes vs bf16
(2.1 MB/core) and enables the PE DoubleRow perf mode (K=256 per
instruction), halving feats stream cycles.

Per core (xT panels [128, 4dc, 1024] fp8, 4 panels):
  1. psum_f[16,1024] = 2 DoubleRow matmuls (K=2x128) per 512-half
     (W cols padded 10->16: dual-fp8 ldweights needs M %% 16 == 0)
  2. E[10,512] = Exp(psum_f + b) per half on ScalarE (bf16)
  3. colsum: per-half [10,8] selector matmul accumulates col-sums of E
     into row g of psum_o[8,512]  (g = batch elem of that half)
  4. gold: ONE fused DVE tensor_tensor_reduce per panel:
     (psum_f * onehot) summed along free dim -> gold_parts[:, p]
     (no PE stream, no full-size scratch: broadcast dummy out)
Host: sum ln(out_c) - sum(out_g) - sum b[tags]  in f64.

DMA: the 4 x-panels are alone on SWDGE q0 (16-engine spread, 4KB
descriptors run back-to-back at 26.4 GB/s/engine; sequential panel
completion lets the PE chase the stream).  oht rides the scalar HWDGE
queue, wt/gsel/b10 the sync HWDGE queue, all overlapped with SWDGE
descriptor-gen.  Act-table prefetch + zero warmup matmuls keep the
Scalar/PE pipelines hot before panel 0 lands.

Non-conforming inputs (different transitions pattern / mask / tag range)
fall back to a faithful numpy port of the reference.
"""

from contextlib import ExitStack

import numpy as np

N_CORES = 8
B, S, D = 64, 512, 512
T = 12
NT = 10          # tags that can actually appear / participate in the LSE
START, STOP = 10, 11
NEG = -10000.0
XSCALE = 8.0     # x/8, W*8 in fp8 e4m3
NTP = 16         # W cols padded to 16: dual-fp8 ldweights needs M % 16 == 0
BS = B // N_CORES          # batch elements per core
R = BS * S                 # s-rows per core (4096)
N_GROUPS = 8               # 512-row groups per core (one batch element each)
GROUP = R // N_GROUPS      # 512
PANEL_COLS = [512, 1024, 1024, 1024, 512]   # asym: small first (early PE
N_PANELS = len(PANEL_COLS)                  # start) and small last (short
PANEL_OFF = [sum(PANEL_COLS[:i]) for i in range(N_PANELS)]     # tail)

_NC_CACHE = None


def _build_nc():
    import concourse.bacc as bacc
    import concourse.mybir as mybir
    import concourse.tile as tile

    f32 = mybir.dt.float32
    bf16 = mybir.dt.bfloat16
    f8 = mybir.dt.float8e4
    nc = bacc.Bacc("TRN2", target_bir_lowering=False, num_swdge_queues=4)

    # panel-major layout: per panel, each SBUF partition reads one
    # contiguous 4KB run -> few fat DMA descriptors
    xt_d = nc.dram_tensor("xt", [128 * 4 * R], f8, kind="ExternalInput")
    oht_d = nc.dram_tensor("oht", [NT, R], bf16, kind="ExternalInput")
    wt_d = nc.dram_tensor("wt", [128, 4, NTP], f8, kind="ExternalInput")
    gsel_d = nc.dram_tensor("gsel", [NT, N_GROUPS * N_GROUPS], bf16,
                            kind="ExternalInput")
    b10_d = nc.dram_tensor("b10", [NT, 1], f32, kind="ExternalInput")
    out_c_d = nc.dram_tensor("out_c", [N_GROUPS, GROUP], f32, kind="ExternalOutput")
    out_g_d = nc.dram_tensor("out_g", [NT, N_PANELS], f32, kind="ExternalOutput")

    with tile.TileContext(nc) as tc, ExitStack() as ctx:
        consts = ctx.enter_context(tc.tile_pool(name="consts", bufs=1))
        xtp = ctx.enter_context(tc.tile_pool(name="xtp", bufs=4))
        epool = ctx.enter_context(tc.tile_pool(name="epool", bufs=2))
        fin = ctx.enter_context(tc.tile_pool(name="fin", bufs=1))
        pf = ctx.enter_context(tc.tile_pool(name="pf", bufs=3, space="PSUM"))
        pacc = ctx.enter_context(tc.tile_pool(name="pacc", bufs=1, space="PSUM"))

        # x panels alone on SWDGE q0, issued first: panel p completes
        # before p+1 so the PE chases the stream
        xt_tiles = []
        off = 0
        for p in range(N_PANELS):
            cols = PANEL_COLS[p]
            xt_sb = xtp.tile([128, 4, cols], f8, name=f"xt_sb{p}", bufs=1)
            nc.gpsimd.dma_start(
                out=xt_sb,
                in_=xt_d[off : off + 128 * 4 * cols].rearrange(
                    "(p c m) -> p c m", p=128, c=4, m=cols
                ),
            )
            xt_tiles.append(xt_sb)
            off += 128 * 4 * cols

        # small consts on the sync HWDGE queue (gen runs parallel to SWDGE;
        # HWDGE swizzles descriptors across all 16 engines).  wt first:
        # the panel-0 feats matmuls need it.
        wt_sb = consts.tile([128, 4, NTP], f8)
        nc.sync.dma_start(out=wt_sb, in_=wt_d[:, :, :])
        oht_sb = consts.tile([NT, R], bf16)
        nc.sync.dma_start(out=oht_sb, in_=oht_d[:, :])
        gsel_sb = consts.tile([NT, N_GROUPS * N_GROUPS], bf16)
        nc.sync.dma_start(out=gsel_sb, in_=gsel_d[:, :])
        b10_sb = consts.tile([NT, 1], f32)
        nc.sync.dma_start(out=b10_sb, in_=b10_d[:, :])

        # scalar-engine Exp table prefetch (first EXP otherwise pays a
        # 1.3us ACT_TABLE_LOAD on the critical path)
        tiny = consts.tile([1, 1], f32, tag="tiny")
        nc.scalar.memzero(tiny)
        tiny_o = consts.tile([1, 1], f32, tag="tiny_o")
        nc.scalar.activation(tiny_o, tiny, mybir.ActivationFunctionType.Exp)

        # PE clock warmup: small all-zero DoubleRow matmuls while DMA
        # streams (cheap memset so they can start early; few enough that
        # they never delay the panel-0 feats matmuls)
        warm = consts.tile([128, 2, 128], f8, tag="warm")
        nc.scalar.memzero(warm)
        psum_w = pacc.tile([NTP, 128], f32, tag="warm")
        for _ in range(24):
            nc.tensor.matmul(psum_w, lhsT=warm[:, :, 0:NTP], rhs=warm,
                             start=True, stop=True,
                             perf_mode=mybir.MatmulPerfMode.DoubleRow)

        psum_o = pacc.tile([N_GROUPS, GROUP], f32, tag="osum")
        gold_parts = fin.tile([NT, N_PANELS], f32, tag="gold")

        n_halves = R // GROUP    # 8 accumulate groups total
        for p in range(N_PANELS):
            xt_sb = xt_tiles[p]
            cols = PANEL_COLS[p]
            coff = PANEL_OFF[p]
            nh = cols // GROUP or 1
            hw = min(cols, GROUP)
            psum_fx = pf.tile([NTP, 1024], f32)
            psum_f = psum_fx[:, 0:cols]
            for h in range(nh):     # matmul out must stay within one PSUM bank
                for j in range(2):  # DoubleRow: dc pair (2j, 2j+1), K=256
                    nc.tensor.matmul(
                        psum_f[0:NTP, h * hw : (h + 1) * hw],
                        lhsT=wt_sb[:, 2 * j : 2 * j + 2, :],
                        rhs=xt_sb[:, 2 * j : 2 * j + 2, h * hw : (h + 1) * hw],
                        start=(j == 0),
                        stop=(j == 1),
                        perf_mode=mybir.MatmulPerfMode.DoubleRow,
                    )

            e_sbx = epool.tile([NT, 1024], bf16)
            e_sb = e_sbx[:, 0:cols]
            nc.scalar.activation(   # one op per panel: halves pay 2x fixed cost
                e_sb, psum_f[0:NT, :],
                mybir.ActivationFunctionType.Exp,
                bias=b10_sb[:, :],
            )
            for h in range(nh):
                g = (coff + h * hw) // GROUP
                nc.tensor.matmul(
                    psum_o,
                    lhsT=gsel_sb[:, N_GROUPS * g : N_GROUPS * (g + 1)],
                    rhs=e_sb[:, h * hw : (h + 1) * hw],
                    start=(g == 0),
                    stop=(g == n_halves - 1),
                )

            # gold partials: fused (psum_f * onehot) + free-dim sum on DVE
            gwx = epool.tile([NT, 1024], bf16, tag=f"gw{p % 2}")
            gw = gwx[:, 0:cols]
            nc.vector.affine_mul_reduce(
                out=gw,
                accum_out=gold_parts[:, p : p + 1],
                in0=psum_f[0:NT, :],
                in1=oht_sb[:, coff : coff + cols],
                scale=1.0,
                bias=0.0,
            )

        # split the result copy across Scalar/Vector and the store across
        # both HWDGE queues so the tail serial chain is halved
        HG = GROUP // 2
        out_sbA = fin.tile([N_GROUPS, HG], f32)
        out_sbB = fin.tile([N_GROUPS, HG], f32)
        nc.scalar.copy(out=out_sbA, in_=psum_o[:, 0:HG])
        nc.vector.tensor_copy(out=out_sbB, in_=psum_o[:, HG:GROUP])
        nc.sync.dma_start(out=out_c_d[:, 0:HG], in_=out_sbA)
        nc.scalar.dma_start(out=out_c_d[:, HG:GROUP], in_=out_sbB)
        nc.scalar.dma_start(out=out_g_d[:, :], in_=gold_parts)

    nc.compile()
    return nc


def _get_nc():
    global _NC_CACHE
    if _NC_CACHE is None:
        _NC_CACHE = _build_nc()
    return _NC_CACHE


def _fast_path_ok(transitions, tags, mask):
    if transitions.shape != (T, T) or tags.min() < 0 or tags.max() >= NT:
        return False
    if not np.all(mask == 1):
        return False
    t2 = np.asarray(transitions, np.float64).copy()
    if not (np.all(t2[START, :] == NEG) and np.all(t2[:, STOP] == NEG)):
        return False
    t2[START, :] = 0.0
    t2[:, STOP] = 0.0
    return bool(np.all(t2 == 0.0))


def _reference_numpy(input_var, W, b, transitions, tags, mask):
    """Faithful float64 port of the reference (fallback only)."""
    x = np.asarray(input_var, np.float64)
    Wf = np.asarray(W, np.float64)
    bf = np.asarray(b, np.float64)
    tr = np.asarray(transitions, np.float64)
    mf = np.asarray(mask, np.float64)
    Bn, Sn, Dn = x.shape
    feats = (x.reshape(-1, Dn) @ Wf.T + bf).reshape(Bn, Sn, -1)
    fv = np.full((Bn, T), NEG)
    fv[:, START] = 0.0
    for t in range(Sn):
        tv = fv[:, None, :] + tr[None] + feats[:, t][:, :, None]
        m = tv.max(axis=2)
        new = m + np.log(np.exp(tv - m[:, :, None]).sum(axis=2))
        fv = new * mf[:, t : t + 1] + fv * (1 - mf[:, t : t + 1])
    fin = fv + tr[STOP][None]
    mm = fin.max(axis=1)
    alpha = mm + np.log(np.exp(fin - mm[:, None]).sum(axis=1))
    score0 = tr[tags[:, 0], START]
    emit = np.take_along_axis(feats[:, :-1], tags[:, :-1, None], axis=2)[..., 0]
    emit_sum = (emit * mf[:, :-1]).sum(axis=1)
    trs = tr[tags[:, 1:], tags[:, :-1]]
    trans_sum = (trs * mf[:, 1:]).sum(axis=1)
    last_idx = np.asarray(mask).sum(axis=1).astype(np.int64) - 1
    last_tags = np.take_along_axis(tags, last_idx[:, None], axis=1)[:, 0]
    last_emit = np.take_along_axis(feats[:, -1], last_tags[:, None], axis=1)[:, 0]
    gold = score0 + emit_sum + trans_sum + tr[STOP, last_tags] + last_emit * mf[:, -1]
    return np.float32((alpha - gold).sum())


def _make_in_maps(input_var, W, b, tags):
    import ml_dtypes

    bf16 = ml_dtypes.bfloat16
    f8 = ml_dtypes.float8_e4m3
    # wt[p, c, j] = XSCALE * W[j, 128c+p] (cols 10-15 zero-padded):
    # one contiguous 64B run per partition
    wpad = np.zeros((NTP, D), np.float32)
    wpad[:NT] = W[:NT] * XSCALE
    wt = np.ascontiguousarray(
        wpad.T.reshape(4, 128, NTP).transpose(1, 0, 2)
    ).astype(f8)
    b10 = np.ascontiguousarray(b[:NT].reshape(NT, 1), np.float32)
    # group selector: block g is [10, 8], col m = 1 iff m == g
    gsel = np.zeros((NT, N_GROUPS * N_GROUPS), np.float32)
    for g in range(N_GROUPS):
        gsel[:, N_GROUPS * g + g] = 1.0
    gsel = gsel.astype(bf16)

    x8 = (input_var.reshape(B * S, D) * (1.0 / XSCALE)).astype(f8)
    onehot = np.zeros((B * S, NT), np.float32)
    onehot[np.arange(B * S), tags.reshape(-1)] = 1.0

    in_maps = []
    for c in range(N_CORES):
        xt = np.ascontiguousarray(x8[c * R : (c + 1) * R].T)        # [512, 4096] f8
        xt4 = xt.reshape(4, 128, R)        # [dc, p, s]
        parts = []
        for p in range(N_PANELS):
            sl = xt4[:, :, PANEL_OFF[p] : PANEL_OFF[p] + PANEL_COLS[p]]
            parts.append(np.ascontiguousarray(sl.transpose(1, 0, 2)).reshape(-1))
        xtp = np.concatenate(parts)        # per-panel [p, dc, s] contiguous runs
        oht = np.ascontiguousarray(onehot[c * R : (c + 1) * R].T).astype(bf16)
        in_maps.append(
            {"xt": xtp, "oht": oht, "wt": wt, "gsel": gsel, "b10": b10}
        )
    return in_maps


def kernel(input_var, W, b, transitions, tags, mask):
    from concourse.bass_utils import run_bass_kernel_spmd

    input_var = np.asarray(input_var)
    W = np.asarray(W)
    b = np.asarray(b)
    transitions = np.asarray(transitions)
    tags = np.asarray(tags)
    mask = np.asarray(mask)

    if not _fast_path_ok(transitions, tags, mask):
        return _reference_numpy(input_var, W, b, transitions, tags, mask)

    nc = _get_nc()
    in_maps = _make_in_maps(input_var, W, b, tags)
    res = run_bass_kernel_spmd(nc, in_maps, list(range(N_CORES)))

    total = np.float64(0.0)
    for c in range(N_CORES):
        csum = np.asarray(res.results[c]["out_c"], np.float64)   # [8, 512]
        gsum = np.asarray(res.results[c]["out_g"], np.float64)   # [10, 4]
        total += np.log(csum).sum() - gsum.sum()
    total -= np.asarray(b, np.float64)[tags].sum()   # gold bias term, host-side
    return np.float32(total)


# revision 17
# speedup vs baseline: 1.0126x; 1.0027x over previous
"""Trainium2 Bass kernel for the DecoderCRF loss (B=64, S=512, D=512, T=12).

Math
----
reference loss = sum_b [ logZ_b - gold_b ] with feats = x @ W.T + b.

For the transitions matrix this problem ships (row START == -1e4, col
STOP == -1e4, everything else 0) and an all-ones mask, the forward
recursion collapses exactly (verified vs a float64 port of the reference):

    logZ_b  = sum_t log( sum_{j=0..9} exp(feats[b,t,j]) )
    gold_b  = sum_t feats[b,t,tags[b,t]]

Layout strategy (v4)
--------------------
Data-parallel over batch: core c handles batch elements 8c..8c+7
(R = 4096 s-rows per core).

fp8 on the input side: host ships x/8 and W*8 in e4m3 (product is exactly
x*W; validated 3.7e-5 rel end-to-end).  Halves DMA bytes vs bf16
(2.1 MB/core) and enables the PE DoubleRow perf mode (K=256 per
instruction), halving feats stream cycles.

Per core (xT panels [128, 4dc, 1024] fp8, 4 panels):
  1. psum_f[16,1024] = 2 DoubleRow matmuls (K=2x128) per 512-half
     (W cols padded 10->16: dual-fp8 ldweights needs M %% 16 == 0)
  2. E[10,512] = Exp(psum_f + b) per half on ScalarE (bf16)
  3. colsum: per-half [10,8] selector matmul accumulates col-sums of E
     into row g of psum_o[8,512]  (g = batch elem of that half)
  4. gold: ONE fused DVE tensor_tensor_reduce per panel:
     (psum_f * onehot) summed along free dim -> gold_parts[:, p]
     (no PE stream, no full-size scratch: broadcast dummy out)
Host: sum ln(out_c) - sum(out_g) - sum b[tags]  in f64.

DMA: the 4 x-panels are alone on SWDGE q0 (16-engine spread, 4KB
descriptors run back-to-back at 26.4 GB/s/engine; sequential panel
completion lets the PE chase the stream).  oht rides the scalar HWDGE
queue, wt/gsel/b10 the sync HWDGE queue, all overlapped with SWDGE
descriptor-gen.  Act-table prefetch + zero warmup matmuls keep the
Scalar/PE pipelines hot before panel 0 lands.

Non-conforming inputs (different transitions pattern / mask / tag range)
fall back to a faithful numpy port of the reference.
"""

from contextlib import ExitStack

import numpy as np

N_CORES = 8
B, S, D = 64, 512, 512
T = 12
NT = 10          # tags that can actually appear / participate in the LSE
START, STOP = 10, 11
NEG = -10000.0
XSCALE = 8.0     # x/8, W*8 in fp8 e4m3
NTP = 16         # W cols padded to 16: dual-fp8 ldweights needs M % 16 == 0
BS = B // N_CORES          # batch elements per core
R = BS * S                 # s-rows per core (4096)
N_GROUPS = 8               # 512-row groups per core (one batch element each)
GROUP = R // N_GROUPS      # 512
PANEL_COLS = [512, 1024, 1024, 1024, 512]   # asym: small first (early PE
N_PANELS = len(PANEL_COLS)                  # start) and small last (short
PANEL_OFF = [sum(PANEL_COLS[:i]) for i in range(N_PANELS)]     # tail)

_NC_CACHE = None


def _build_nc():
    import concourse.bacc as bacc
    import concourse.mybir as mybir
    import concourse.tile as tile

    f32 = mybir.dt.float32
    bf16 = mybir.dt.bfloat16
    f8 = mybir.dt.float8e4
    nc = bacc.Bacc("TRN2", target_bir_lowering=False, num_swdge_queues=4)

    # panel-major layout: per panel, each SBUF partition reads one
    # contiguous 4KB run -> few fat DMA descriptors
    xt_d = nc.dram_tensor("xt", [128 * 4 * R], f8, kind="ExternalInput")
    oht_d = nc.dram_tensor("oht", [NT, R], bf16, kind="ExternalInput")
    wt_d = nc.dram_tensor("wt", [128, 4, NTP], f8, kind="ExternalInput")
    gsel_d = nc.dram_tensor("gsel", [NT, 4 * N_GROUPS], bf16,
                            kind="ExternalInput")
    b10_d = nc.dram_tensor("b10", [NT, 1], f32, kind="ExternalInput")
    out_c_d = nc.dram_tensor("out_c", [N_GROUPS, GROUP], f32, kind="ExternalOutput")
    out_g_d = nc.dram_tensor("out_g", [NT, N_PANELS], f32, kind="ExternalOutput")

    with tile.TileContext(nc) as tc, ExitStack() as ctx:
        consts = ctx.enter_context(tc.tile_pool(name="consts", bufs=1))
        xtp = ctx.enter_context(tc.tile_pool(name="xtp", bufs=4))
        epool = ctx.enter_context(tc.tile_pool(name="epool", bufs=2))
        fin = ctx.enter_context(tc.tile_pool(name="fin", bufs=1))
        pf = ctx.enter_context(tc.tile_pool(name="pf", bufs=3, space="PSUM"))
        pacc = ctx.enter_context(tc.tile_pool(name="pacc", bufs=1, space="PSUM"))

        # x panels alone on SWDGE q0, issued first: panel p completes
        # before p+1 so the PE chases the stream
        xt_tiles = []
        off = 0
        for p in range(N_PANELS):
            cols = PANEL_COLS[p]
            xt_sb = xtp.tile([128, 4, cols], f8, name=f"xt_sb{p}", bufs=1)
            nc.gpsimd.dma_start(
                out=xt_sb,
                in_=xt_d[off : off + 128 * 4 * cols].rearrange(
                    "(p c m) -> p c m", p=128, c=4, m=cols
                ),
            )
            xt_tiles.append(xt_sb)
            off += 128 * 4 * cols

        # small consts on the sync HWDGE queue (gen runs parallel to SWDGE;
        # HWDGE swizzles descriptors across all 16 engines).  wt first:
        # the panel-0 feats matmuls need it.
        wt_sb = consts.tile([128, 4, NTP], f8)
        nc.sync.dma_start(out=wt_sb, in_=wt_d[:, :, :])
        oht_sb = consts.tile([NT, R], bf16)
        nc.sync.dma_start(out=oht_sb, in_=oht_d[:, :])
        gsel_sb = consts.tile([NT, 4 * N_GROUPS], bf16)
        nc.sync.dma_start(out=gsel_sb, in_=gsel_d[:, :])
        b10_sb = consts.tile([NT, 1], f32)
        nc.sync.dma_start(out=b10_sb, in_=b10_d[:, :])

        # scalar-engine Exp table prefetch (first EXP otherwise pays a
        # 1.3us ACT_TABLE_LOAD on the critical path)
        tiny = consts.tile([1, 1], f32, tag="tiny")
        nc.scalar.memzero(tiny)
        tiny_o = consts.tile([1, 1], f32, tag="tiny_o")
        nc.scalar.activation(tiny_o, tiny, mybir.ActivationFunctionType.Exp)

        # two colsum accumulators: groups 0-3 finish mid-stream so their
        # copy+store overlaps the remaining panels; only groups 4-7 (and
        # gold) sit on the end-of-kernel critical path
        psum_oA = pacc.tile([4, GROUP], f32, tag="osumA")
        psum_oB = pacc.tile([4, GROUP], f32, tag="osumB")
        gold_parts = fin.tile([NT, N_PANELS], f32, tag="gold")

        n_halves = R // GROUP    # 8 accumulate groups total
        for p in range(N_PANELS):
            xt_sb = xt_tiles[p]
            cols = PANEL_COLS[p]
            coff = PANEL_OFF[p]
            nh = cols // GROUP or 1
            hw = min(cols, GROUP)
            psum_fx = pf.tile([NTP, 1024], f32)
            psum_f = psum_fx[:, 0:cols]
            for h in range(nh):     # matmul out must stay within one PSUM bank
                for j in range(2):  # DoubleRow: dc pair (2j, 2j+1), K=256
                    nc.tensor.matmul(
                        psum_f[0:NTP, h * hw : (h + 1) * hw],
                        lhsT=wt_sb[:, 2 * j : 2 * j + 2, :],
                        rhs=xt_sb[:, 2 * j : 2 * j + 2, h * hw : (h + 1) * hw],
                        start=(j == 0),
                        stop=(j == 1),
                        perf_mode=mybir.MatmulPerfMode.DoubleRow,
                    )

            e_sbx = epool.tile([NT, 1024], bf16)
            e_sb = e_sbx[:, 0:cols]
            nc.scalar.activation(   # one op per panel: halves pay 2x fixed cost
                e_sb, psum_f[0:NT, :],
                mybir.ActivationFunctionType.Exp,
                bias=b10_sb[:, :],
            )
            for h in range(nh):
                g = (coff + h * hw) // GROUP
                half, gi = divmod(g, 4)
                nc.tensor.matmul(
                    psum_oA if half == 0 else psum_oB,
                    lhsT=gsel_sb[:, 4 * g : 4 * (g + 1)],
                    rhs=e_sb[:, h * hw : (h + 1) * hw],
                    start=(gi == 0),
                    stop=(gi == 3),
                )

            # gold partials: fused (psum_f * onehot) + free-dim sum on DVE
            gwx = epool.tile([NT, 1024], bf16, tag=f"gw{p % 2}")
            gw = gwx[:, 0:cols]
            nc.vector.affine_mul_reduce(
                out=gw,
                accum_out=gold_parts[:, p : p + 1],
                in0=psum_f[0:NT, :],
                in1=oht_sb[:, coff : coff + cols],
                scale=1.0,
                bias=0.0,
            )

        out_sbA = fin.tile([4, GROUP], f32)
        nc.scalar.copy(out=out_sbA, in_=psum_oA)
        nc.sync.dma_start(out=out_c_d[0:4, :], in_=out_sbA)
        out_sbB = fin.tile([4, GROUP], f32)
        nc.scalar.copy(out=out_sbB, in_=psum_oB)
        nc.sync.dma_start(out=out_c_d[4:N_GROUPS, :], in_=out_sbB)
        nc.gpsimd.dma_start(out=out_g_d[:, :], in_=gold_parts)

    nc.compile()
    return nc


def _get_nc():
    global _NC_CACHE
    if _NC_CACHE is None:
        _NC_CACHE = _build_nc()
    return _NC_CACHE


def _fast_path_ok(transitions, tags, mask):
    if transitions.shape != (T, T) or tags.min() < 0 or tags.max() >= NT:
        return False
    if not np.all(mask == 1):
        return False
    t2 = np.asarray(transitions, np.float64).copy()
    if not (np.all(t2[START, :] == NEG) and np.all(t2[:, STOP] == NEG)):
        return False
    t2[START, :] = 0.0
    t2[:, STOP] = 0.0
    return bool(np.all(t2 == 0.0))


def _reference_numpy(input_var, W, b, transitions, tags, mask):
    """Faithful float64 port of the reference (fallback only)."""
    x = np.asarray(input_var, np.float64)
    Wf = np.asarray(W, np.float64)
    bf = np.asarray(b, np.float64)
    tr = np.asarray(transitions, np.float64)
    mf = np.asarray(mask, np.float64)
    Bn, Sn, Dn = x.shape
    feats = (x.reshape(-1, Dn) @ Wf.T + bf).reshape(Bn, Sn, -1)
    fv = np.full((Bn, T), NEG)
    fv[:, START] = 0.0
    for t in range(Sn):
        tv = fv[:, None, :] + tr[None] + feats[:, t][:, :, None]
        m = tv.max(axis=2)
        new = m + np.log(np.exp(tv - m[:, :, None]).sum(axis=2))
        fv = new * mf[:, t : t + 1] + fv * (1 - mf[:, t : t + 1])
    fin = fv + tr[STOP][None]
    mm = fin.max(axis=1)
    alpha = mm + np.log(np.exp(fin - mm[:, None]).sum(axis=1))
    score0 = tr[tags[:, 0], START]
    emit = np.take_along_axis(feats[:, :-1], tags[:, :-1, None], axis=2)[..., 0]
    emit_sum = (emit * mf[:, :-1]).sum(axis=1)
    trs = tr[tags[:, 1:], tags[:, :-1]]
    trans_sum = (trs * mf[:, 1:]).sum(axis=1)
    last_idx = np.asarray(mask).sum(axis=1).astype(np.int64) - 1
    last_tags = np.take_along_axis(tags, last_idx[:, None], axis=1)[:, 0]
    last_emit = np.take_along_axis(feats[:, -1], last_tags[:, None], axis=1)[:, 0]
    gold = score0 + emit_sum + trans_sum + tr[STOP, last_tags] + last_emit * mf[:, -1]
    return np.float32((alpha - gold).sum())


def _make_in_maps(input_var, W, b, tags):
    import ml_dtypes

    bf16 = ml_dtypes.bfloat16
    f8 = ml_dtypes.float8_e4m3
    # wt[p, c, j] = XSCALE * W[j, 128c+p] (cols 10-15 zero-padded):
    # one contiguous 64B run per partition
    wpad = np.zeros((NTP, D), np.float32)
    wpad[:NT] = W[:NT] * XSCALE
    wt = np.ascontiguousarray(
        wpad.T.reshape(4, 128, NTP).transpose(1, 0, 2)
    ).astype(f8)
    b10 = np.ascontiguousarray(b[:NT].reshape(NT, 1), np.float32)
    # group selector: block g is [10, 4], col m = 1 iff m == g % 4
    gsel = np.zeros((NT, 4 * N_GROUPS), np.float32)
    for g in range(N_GROUPS):
        gsel[:, 4 * g + (g % 4)] = 1.0
    gsel = gsel.astype(bf16)

    x8 = (input_var.reshape(B * S, D) * (1.0 / XSCALE)).astype(f8)
    onehot = np.zeros((B * S, NT), np.float32)
    onehot[np.arange(B * S), tags.reshape(-1)] = 1.0

    in_maps = []
    for c in range(N_CORES):
        xt = np.ascontiguousarray(x8[c * R : (c + 1) * R].T)        # [512, 4096] f8
        xt4 = xt.reshape(4, 128, R)        # [dc, p, s]
        parts = []
        for p in range(N_PANELS):
            sl = xt4[:, :, PANEL_OFF[p] : PANEL_OFF[p] + PANEL_COLS[p]]
            parts.append(np.ascontiguousarray(sl.transpose(1, 0, 2)).reshape(-1))
        xtp = np.concatenate(parts)        # per-panel [p, dc, s] contiguous runs
        oht = np.ascontiguousarray(onehot[c * R : (c + 1) * R].T).astype(bf16)
        in_maps.append(
            {"xt": xtp, "oht": oht, "wt": wt, "gsel": gsel, "b10": b10}
        )
    return in_maps


def kernel(input_var, W, b, transitions, tags, mask):
    from concourse.bass_utils import run_bass_kernel_spmd

    input_var = np.asarray(input_var)
    W = np.asarray(W)
    b = np.asarray(b)
    transitions = np.asarray(transitions)
    tags = np.asarray(tags)
    mask = np.asarray(mask)

    if not _fast_path_ok(transitions, tags, mask):
        return _reference_numpy(input_var, W, b, transitions, tags, mask)

    nc = _get_nc()
    in_maps = _make_in_maps(input_var, W, b, tags)
    res = run_bass_kernel_spmd(nc, in_maps, list(range(N_CORES)))

    total = np.float64(0.0)
    for c in range(N_CORES):
        csum = np.asarray(res.results[c]["out_c"], np.float64)   # [8, 512]
        gsum = np.asarray(res.results[c]["out_g"], np.float64)   # [10, 4]
        total += np.log(csum).sum() - gsum.sum()
    total -= np.asarray(b, np.float64)[tags].sum()   # gold bias term, host-side
    return np.float32(total)


# revision 18
# speedup vs baseline: 1.0162x; 1.0036x over previous
"""Trainium2 Bass kernel for the DecoderCRF loss (B=64, S=512, D=512, T=12).

Math
----
reference loss = sum_b [ logZ_b - gold_b ] with feats = x @ W.T + b.

For the transitions matrix this problem ships (row START == -1e4, col
STOP == -1e4, everything else 0) and an all-ones mask, the forward
recursion collapses exactly (verified vs a float64 port of the reference):

    logZ_b  = sum_t log( sum_{j=0..9} exp(feats[b,t,j]) )
    gold_b  = sum_t feats[b,t,tags[b,t]]

Layout strategy (v4)
--------------------
Data-parallel over batch: core c handles batch elements 8c..8c+7
(R = 4096 s-rows per core).

fp8 on the input side: host ships x/8 and W*8 in e4m3 (product is exactly
x*W; validated 3.7e-5 rel end-to-end).  Halves DMA bytes vs bf16
(2.1 MB/core) and enables the PE DoubleRow perf mode (K=256 per
instruction), halving feats stream cycles.

Per core (xT panels [128, 4dc, 1024] fp8, 4 panels):
  1. psum_f[16,1024] = 2 DoubleRow matmuls (K=2x128) per 512-half
     (W cols padded 10->16: dual-fp8 ldweights needs M %% 16 == 0)
  2. E[10,512] = Exp(psum_f + b) per half on ScalarE (bf16)
  3. colsum: per-half [10,8] selector matmul accumulates col-sums of E
     into row g of psum_o[8,512]  (g = batch elem of that half)
  4. gold: ONE fused DVE tensor_tensor_reduce per panel:
     (psum_f * onehot) summed along free dim -> gold_parts[:, p]
     (no PE stream, no full-size scratch: broadcast dummy out)
Host: sum ln(out_c) - sum(out_g) - sum b[tags]  in f64.

DMA: the 4 x-panels are alone on SWDGE q0 (16-engine spread, 4KB
descriptors run back-to-back at 26.4 GB/s/engine; sequential panel
completion lets the PE chase the stream).  oht rides the scalar HWDGE
queue, wt/gsel/b10 the sync HWDGE queue, all overlapped with SWDGE
descriptor-gen.  Act-table prefetch + zero warmup matmuls keep the
Scalar/PE pipelines hot before panel 0 lands.

Non-conforming inputs (different transitions pattern / mask / tag range)
fall back to a faithful numpy port of the reference.
"""

from contextlib import ExitStack

import numpy as np

N_CORES = 8
B, S, D = 64, 512, 512
T = 12
NT = 10          # tags that can actually appear / participate in the LSE
START, STOP = 10, 11
NEG = -10000.0
XSCALE = 8.0     # x/8, W*8 in fp8 e4m3
NTP = 16         # W cols padded to 16: dual-fp8 ldweights needs M % 16 == 0
BS = B // N_CORES          # batch elements per core
R = BS * S                 # s-rows per core (4096)
N_GROUPS = 8               # 512-row groups per core (one batch element each)
GROUP = R // N_GROUPS      # 512
PANEL_COLS = [512, 1024, 1024, 1024, 512]   # asym: small first (early PE
N_PANELS = len(PANEL_COLS)                  # start) and small last (short
PANEL_OFF = [sum(PANEL_COLS[:i]) for i in range(N_PANELS)]     # tail)

_NC_CACHE = None


def _build_nc():
    import concourse.bacc as bacc
    import concourse.mybir as mybir
    import concourse.tile as tile

    f32 = mybir.dt.float32
    bf16 = mybir.dt.bfloat16
    f8 = mybir.dt.float8e4
    nc = bacc.Bacc("TRN2", target_bir_lowering=False, num_swdge_queues=4)

    # panel-major layout: per panel, each SBUF partition reads one
    # contiguous 4KB run -> few fat DMA descriptors
    xt_d = nc.dram_tensor("xt", [128 * 4 * R], f8, kind="ExternalInput")
    oht_d = nc.dram_tensor("oht", [NT, R], bf16, kind="ExternalInput")
    wt_d = nc.dram_tensor("wt", [128, 4, NTP], f8, kind="ExternalInput")
    gsel_d = nc.dram_tensor("gsel", [NT, 4 * N_GROUPS], bf16,
                            kind="ExternalInput")
    b10_d = nc.dram_tensor("b10", [NT, 1], f32, kind="ExternalInput")
    out_c_d = nc.dram_tensor("out_c", [N_GROUPS, GROUP], f32, kind="ExternalOutput")
    out_g_d = nc.dram_tensor("out_g", [NT, N_PANELS], f32, kind="ExternalOutput")

    with tile.TileContext(nc) as tc, ExitStack() as ctx:
        consts = ctx.enter_context(tc.tile_pool(name="consts", bufs=1))
        xtp = ctx.enter_context(tc.tile_pool(name="xtp", bufs=4))
        epool = ctx.enter_context(tc.tile_pool(name="epool", bufs=2))
        fin = ctx.enter_context(tc.tile_pool(name="fin", bufs=1))
        pf = ctx.enter_context(tc.tile_pool(name="pf", bufs=3, space="PSUM"))
        pacc = ctx.enter_context(tc.tile_pool(name="pacc", bufs=1, space="PSUM"))

        # x panels alone on SWDGE q0, issued first: panel p completes
        # before p+1 so the PE chases the stream
        xt_tiles = []
        off = 0
        for p in range(N_PANELS):
            cols = PANEL_COLS[p]
            xt_sb = xtp.tile([128, 4, cols], f8, name=f"xt_sb{p}", bufs=1)
            nc.gpsimd.dma_start(
                out=xt_sb,
                in_=xt_d[off : off + 128 * 4 * cols].rearrange(
                    "(p c m) -> p c m", p=128, c=4, m=cols
                ),
            )
            xt_tiles.append(xt_sb)
            off += 128 * 4 * cols

        # small consts on the sync HWDGE queue (gen runs parallel to SWDGE;
        # the ring dispatches ops in order with ~1us latency each, so order
        # by first-use: b10 (exp p0), wt (feats p0), gsel (E-mat g0),
        # oht (AMR p0)
        b10_sb = consts.tile([NT, 1], f32)
        nc.sync.dma_start(out=b10_sb, in_=b10_d[:, :])
        wt_sb = consts.tile([128, 4, NTP], f8)
        nc.sync.dma_start(out=wt_sb, in_=wt_d[:, :, :])
        gsel_sb = consts.tile([NT, 4 * N_GROUPS], bf16)
        nc.sync.dma_start(out=gsel_sb, in_=gsel_d[:, :])
        oht_sb = consts.tile([NT, R], bf16)
        nc.sync.dma_start(out=oht_sb, in_=oht_d[:, :])

        # scalar-engine Exp table prefetch (first EXP otherwise pays a
        # 1.3us ACT_TABLE_LOAD on the critical path)
        tiny = consts.tile([1, 1], f32, tag="tiny")
        nc.scalar.memzero(tiny)
        tiny_o = consts.tile([1, 1], f32, tag="tiny_o")
        nc.scalar.activation(tiny_o, tiny, mybir.ActivationFunctionType.Exp)

        # two colsum accumulators: groups 0-3 finish mid-stream so their
        # copy+store overlaps the remaining panels; only groups 4-7 (and
        # gold) sit on the end-of-kernel critical path
        psum_oA = pacc.tile([4, GROUP], f32, tag="osumA")
        psum_oB = pacc.tile([4, GROUP], f32, tag="osumB")

        # PE clock-ramp warmup: big-N bf16 matmuls (small-N ones do not
        # engage the ramp); they scribble into psum_oA, which g0's
        # start=True later resets.  memzero on Scalar so the gpsimd/vector
        # balancer cannot put it in front of the SWDGE descriptor-gens.
        warm = consts.tile([128, GROUP], bf16, tag="warm")
        nc.scalar.memzero(warm)
        for _ in range(6):
            nc.tensor.matmul(psum_oA, lhsT=warm[:, 0:4], rhs=warm,
                             start=True, stop=True)
        gold_parts = fin.tile([NT, N_PANELS], f32, tag="gold")

        n_halves = R // GROUP    # 8 accumulate groups total
        for p in range(N_PANELS):
            xt_sb = xt_tiles[p]
            cols = PANEL_COLS[p]
            coff = PANEL_OFF[p]
            nh = cols // GROUP or 1
            hw = min(cols, GROUP)
            psum_fx = pf.tile([NTP, 1024], f32)
            psum_f = psum_fx[:, 0:cols]
            for h in range(nh):     # matmul out must stay within one PSUM bank
                for j in range(2):  # DoubleRow: dc pair (2j, 2j+1), K=256
                    nc.tensor.matmul(
                        psum_f[0:NTP, h * hw : (h + 1) * hw],
                        lhsT=wt_sb[:, 2 * j : 2 * j + 2, :],
                        rhs=xt_sb[:, 2 * j : 2 * j + 2, h * hw : (h + 1) * hw],
                        start=(j == 0),
                        stop=(j == 1),
                        perf_mode=mybir.MatmulPerfMode.DoubleRow,
                    )

            e_sbx = epool.tile([NT, 1024], bf16)
            e_sb = e_sbx[:, 0:cols]
            nc.scalar.activation(   # one op per panel: halves pay 2x fixed cost
                e_sb, psum_f[0:NT, :],
                mybir.ActivationFunctionType.Exp,
                bias=b10_sb[:, :],
            )
            for h in range(nh):
                g = (coff + h * hw) // GROUP
                half, gi = divmod(g, 4)
                nc.tensor.matmul(
                    psum_oA if half == 0 else psum_oB,
                    lhsT=gsel_sb[:, 4 * g : 4 * (g + 1)],
                    rhs=e_sb[:, h * hw : (h + 1) * hw],
                    start=(gi == 0),
                    stop=(gi == 3),
                )

            # gold partials: fused (psum_f * onehot) + free-dim sum on DVE
            gwx = epool.tile([NT, 1024], bf16, tag=f"gw{p % 2}")
            gw = gwx[:, 0:cols]
            nc.vector.affine_mul_reduce(
                out=gw,
                accum_out=gold_parts[:, p : p + 1],
                in0=psum_f[0:NT, :],
                in1=oht_sb[:, coff : coff + cols],
                scale=1.0,
                bias=0.0,
            )
            if p == 2:   # groups 0-3 complete: store them under the stream
                out_sbA = fin.tile([4, GROUP], f32, tag="outA")
                nc.scalar.copy(out=out_sbA, in_=psum_oA)
                nc.sync.dma_start(out=out_c_d[0:4, :], in_=out_sbA)

        out_sbB = fin.tile([4, GROUP], f32)
        nc.scalar.copy(out=out_sbB, in_=psum_oB)
        nc.sync.dma_start(out=out_c_d[4:N_GROUPS, :], in_=out_sbB)
        nc.sync.dma_start(out=out_g_d[:, :], in_=gold_parts)

    nc.compile()
    return nc


def _get_nc():
    global _NC_CACHE
    if _NC_CACHE is None:
        _NC_CACHE = _build_nc()
    return _NC_CACHE


def _fast_path_ok(transitions, tags, mask):
    if transitions.shape != (T, T) or tags.min() < 0 or tags.max() >= NT:
        return False
    if not np.all(mask == 1):
        return False
    t2 = np.asarray(transitions, np.float64).copy()
    if not (np.all(t2[START, :] == NEG) and np.all(t2[:, STOP] == NEG)):
        return False
    t2[START, :] = 0.0
    t2[:, STOP] = 0.0
    return bool(np.all(t2 == 0.0))


def _reference_numpy(input_var, W, b, transitions, tags, mask):
    """Faithful float64 port of the reference (fallback only)."""
    x = np.asarray(input_var, np.float64)
    Wf = np.asarray(W, np.float64)
    bf = np.asarray(b, np.float64)
    tr = np.asarray(transitions, np.float64)
    mf = np.asarray(mask, np.float64)
    Bn, Sn, Dn = x.shape
    feats = (x.reshape(-1, Dn) @ Wf.T + bf).reshape(Bn, Sn, -1)
    fv = np.full((Bn, T), NEG)
    fv[:, START] = 0.0
    for t in range(Sn):
        tv = fv[:, None, :] + tr[None] + feats[:, t][:, :, None]
        m = tv.max(axis=2)
        new = m + np.log(np.exp(tv - m[:, :, None]).sum(axis=2))
        fv = new * mf[:, t : t + 1] + fv * (1 - mf[:, t : t + 1])
    fin = fv + tr[STOP][None]
    mm = fin.max(axis=1)
    alpha = mm + np.log(np.exp(fin - mm[:, None]).sum(axis=1))
    score0 = tr[tags[:, 0], START]
    emit = np.take_along_axis(feats[:, :-1], tags[:, :-1, None], axis=2)[..., 0]
    emit_sum = (emit * mf[:, :-1]).sum(axis=1)
    trs = tr[tags[:, 1:], tags[:, :-1]]
    trans_sum = (trs * mf[:, 1:]).sum(axis=1)
    last_idx = np.asarray(mask).sum(axis=1).astype(np.int64) - 1
    last_tags = np.take_along_axis(tags, last_idx[:, None], axis=1)[:, 0]
    last_emit = np.take_along_axis(feats[:, -1], last_tags[:, None], axis=1)[:, 0]
    gold = score0 + emit_sum + trans_sum + tr[STOP, last_tags] + last_emit * mf[:, -1]
    return np.float32((alpha - gold).sum())


def _make_in_maps(input_var, W, b, tags):
    import ml_dtypes

    bf16 = ml_dtypes.bfloat16
    f8 = ml_dtypes.float8_e4m3
    # wt[p, c, j] = XSCALE * W[j, 128c+p] (cols 10-15 zero-padded):
    # one contiguous 64B run per partition
    wpad = np.zeros((NTP, D), np.float32)
    wpad[:NT] = W[:NT] * XSCALE
    wt = np.ascontiguousarray(
        wpad.T.reshape(4, 128, NTP).transpose(1, 0, 2)
    ).astype(f8)
    b10 = np.ascontiguousarray(b[:NT].reshape(NT, 1), np.float32)
    # group selector: block g is [10, 4], col m = 1 iff m == g % 4
    gsel = np.zeros((NT, 4 * N_GROUPS), np.float32)
    for g in range(N_GROUPS):
        gsel[:, 4 * g + (g % 4)] = 1.0
    gsel = gsel.astype(bf16)

    x8 = (input_var.reshape(B * S, D) * (1.0 / XSCALE)).astype(f8)
    onehot = np.zeros((B * S, NT), np.float32)
    onehot[np.arange(B * S), tags.reshape(-1)] = 1.0

    in_maps = []
    for c in range(N_CORES):
        xt = np.ascontiguousarray(x8[c * R : (c + 1) * R].T)        # [512, 4096] f8
        xt4 = xt.reshape(4, 128, R)        # [dc, p, s]
        parts = []
        for p in range(N_PANELS):
            sl = xt4[:, :, PANEL_OFF[p] : PANEL_OFF[p] + PANEL_COLS[p]]
            parts.append(np.ascontiguousarray(sl.transpose(1, 0, 2)).reshape(-1))
        xtp = np.concatenate(parts)        # per-panel [p, dc, s] contiguous runs
        oht = np.ascontiguousarray(onehot[c * R : (c + 1) * R].T).astype(bf16)
        in_maps.append(
            {"xt": xtp, "oht": oht, "wt": wt, "gsel": gsel, "b10": b10}
        )
    return in_maps


def kernel(input_var, W, b, transitions, tags, mask):
    from concourse.bass_utils import run_bass_kernel_spmd

    input_var = np.asarray(input_var)
    W = np.asarray(W)
    b = np.asarray(b)
    transitions = np.asarray(transitions)
    tags = np.asarray(tags)
    mask = np.asarray(mask)

    if not _fast_path_ok(transitions, tags, mask):
        return _reference_numpy(input_var, W, b, transitions, tags, mask)

    nc = _get_nc()
    in_maps = _make_in_maps(input_var, W, b, tags)
    res = run_bass_kernel_spmd(nc, in_maps, list(range(N_CORES)))

    total = np.float64(0.0)
    for c in range(N_CORES):
        csum = np.asarray(res.results[c]["out_c"], np.float64)   # [8, 512]
        gsum = np.asarray(res.results[c]["out_g"], np.float64)   # [10, 4]
        total += np.log(csum).sum() - gsum.sum()
    total -= np.asarray(b, np.float64)[tags].sum()   # gold bias term, host-side
    return np.float32(total)
